# revision 1
# baseline (speedup 1.0000x reference)
"""CrossNet layer (encoder Dense + 4 cross layers) on 8 trn2 NeuronCores.

Pure data parallelism: batch 1024 is split into 8 shards of 128 rows;
encoder weights + tiny cross weights are replicated per core.

Math: with h = x @ W_enc + b_enc, x0 = h, the cross recurrence
    x_{l+1} = x_l + x0 * (x_l @ w_l) + b_l
keeps the closed form x_l = x0 * c_l + B_l with per-row scalar c_l and
H-vector B_l = sum_{j<l} b_j, since
    s_l = x_l @ w_l = c_l * (x0 @ w_l) + B_l @ w_l = c_l * p_l + q_l
    c_{l+1} = c_l * (1 + p_l) + q_l,   c_0 = 1.
So the device only needs the big matmul h, P = x0 @ Wc (Wc = ws^T),
the 4x4 table Q[j,l] = b_j @ w_l (q_l = sum_{j<l} Q[j,l]), a 4-step scan
for c, and out = x0 * c_4 + B_4.

Schedule: x loads first, then 4 x 1MB W chunks in parallel (per-core HBM
rate is chip-contention-bound at ~220GB/s with 8 cores loading replicated
weights) feeding a k-outer matmul loop; the h->h^T->P tail runs as
a per-128-column pipeline across PE/ACT/DVE; f32r matmuls (4x the fp32
rate) via bitcast loads.
"""

import numpy as np

B, D, H, DEPTH = 1024, 1024, 1024, 4
N_CORES = 8
BS = B // N_CORES  # batch rows per core
KT = D // 128      # contraction k-tiles
NT = H // 512      # psum n-tiles

_cache = {}


def _patch_tile_drain(max_waits: int = 1):
    """walrus in this image allows only 1 sync-wait per instruction; the stock
    Tile end-of-kernel drain carries the whole global clock on one SP Drain and
    codegen fails. Split the waits across a chain of SP nops instead."""
    import concourse.tile as tile
    from concourse.vector_clock import ScopedClock
    from concourse import mybir

    if getattr(tile.TileContext, "_drain_patched", False):
        return

    def _drain_and_barrier(self, tick_clock, wait_clock):
        nc = self.nc
        carrier = nc.sync.nop()
        wait_clock.add_sem_waits(
            carrier.ins, ScopedClock({None: tick_clock.global_clock})
        )
        si = carrier.ins.sync_info
        if si is not None and si.on_wait and len(si.on_wait) > max_waits:
            waits = list(si.on_wait)
            carrier.ins.sync_info = mybir.SyncInfo(
                on_wait=waits[:max_waits], on_update=list(si.on_update or [])
            )
            rest = waits[max_waits:]
            while rest:
                extra = nc.sync.nop()
                extra.ins.sync_info = mybir.SyncInfo(
                    on_wait=rest[:max_waits], on_update=[]
                )
                rest = rest[max_waits:]
        nc.sync.drain()

        # exit barrier + sem clears dropped: the NEFF preamble re-inits
        # semaphores on every execution (verified by back-to-back runs), so
        # the ~4us exit butterfly only burns measured time
        assert self.sems is not None
        popped = nc._tile_sem_poison_stack.pop()
        assert popped is self._sem_poison

    tile.TileContext._drain_and_barrier = _drain_and_barrier
    tile.TileContext._drain_patched = True


def _split_multi_waits(nc):
    """walrus here allows only one sync-wait per instruction: move extra waits
    onto same-engine NoOps inserted immediately before the instruction."""
    from concourse import mybir

    for fn in nc.m.functions:
        for bb in fn.blocks:
            out = []
            for inst in bb.instructions:
                si = inst.sync_info
                if si is not None and si.on_wait and len(si.on_wait) > 1:
                    waits = list(si.on_wait)
                    for i, w in enumerate(waits[:-1]):
                        nop = mybir.InstNoOp(name=f"{inst.name}-w{i}", ins=[], outs=[])
                        nop.engine = inst.engine
                        nop.sync_info = mybir.SyncInfo(on_wait=[w], on_update=[])
                        out.append(nop)
                    inst.sync_info = mybir.SyncInfo(
                        on_wait=[waits[-1]], on_update=list(si.on_update or [])
                    )
                out.append(inst)
            bb.instructions[:] = out


def _build(use_f32r=True, split=True):
    from contextlib import ExitStack

    import concourse.bass as bass
    import concourse.tile as tile
    from concourse import mybir

    _patch_tile_drain()

    fp32 = mybir.dt.float32
    f32r = mybir.dt.float32r
    i32 = mybir.dt.int32
    Alu = mybir.AluOpType

    nc = bass.Bass()
    x_in = nc.declare_dram_parameter("x", [BS, D], fp32, isOutput=False)
    w_in = nc.declare_dram_parameter("w", [D, H], fp32, isOutput=False)
    be_in = nc.declare_dram_parameter("be", [1, H], fp32, isOutput=False)
    ws_in = nc.declare_dram_parameter("ws", [DEPTH, H], fp32, isOutput=False)
    bs_in = nc.declare_dram_parameter("bs", [DEPTH, H], fp32, isOutput=False)
    y_out = nc.declare_dram_parameter("y", [BS, H], fp32, isOutput=True)

    with ExitStack() as ctx:
        tc = ctx.enter_context(tile.TileContext(nc))
        cpool = ctx.enter_context(tc.tile_pool(name="const", bufs=1))
        wpool = ctx.enter_context(tc.tile_pool(name="w", bufs=2 * KT))
        iop = ctx.enter_context(tc.tile_pool(name="io", bufs=1))
        xtp = ctx.enter_context(tc.tile_pool(name="xt", bufs=KT))
        htp = ctx.enter_context(tc.tile_pool(name="ht", bufs=KT))
        smp = ctx.enter_context(tc.tile_pool(name="sm", bufs=KT))
        pst = ctx.enter_context(tc.tile_pool(name="pst", bufs=2, space="PSUM"))
        psh = ctx.enter_context(tc.tile_pool(name="psh", bufs=2, space="PSUM"))
        psb = ctx.enter_context(tc.tile_pool(name="psb", bufs=2, space="PSUM"))
        psq = ctx.enter_context(tc.tile_pool(name="psq", bufs=1, space="PSUM"))

        # ---- input DMAs -------------------------------------------------
        x_sb = iop.tile([BS, D], fp32)
        x_dma = nc.sync.dma_start(x_sb[:], x_in[:])
        # small tensors on the ACT HWDGE ring: keeps the SP ring's serial
        # issue budget (~0.65us per dma) for x + the W stream
        be_sb = iop.tile([1, H], f32r if use_f32r else fp32)
        nc.scalar.dma_start(be_sb[:], be_in[:].bitcast(f32r) if use_f32r else be_in[:])
        ws_sb = iop.tile([DEPTH, H], fp32)
        nc.scalar.dma_start(ws_sb[:], ws_in[:])
        bs_sb = iop.tile([DEPTH, H], fp32)
        nc.scalar.dma_start(bs_sb[:], bs_in[:])
        from concourse.tile_rust import add_dep_helper

        # W in 4 x 1MB chunks (two 128-row k-tiles side by side), f32r via
        # bitcast (PE truncates low mantissa bits; measured same numerics as
        # pre-rounded).
        w2 = []
        w_dmas = []
        for c in range(KT // 2):
            wc2 = wpool.tile(
                [128, 2, H], f32r if use_f32r else fp32, tag="wr", name=f"wr{c}"
            )
            src_ap = w_in[c * 256 : (c + 1) * 256, :].rearrange(
                "(a p) h -> p a h", p=128
            )
            if use_f32r:
                src_ap = src_ap.bitcast(f32r)
            dma = nc.sync.dma_start(wc2[:], src_ap)
            # chunk 0 rides with x; the rest wait so x (which gates the
            # x^T transposes) isn't starved by the 4MB W round-robin
            if c > 0:
                add_dep_helper(dma.ins, x_dma.ins, reason="x-first")
            w_dmas.append(dma)
            w2.append(wc2)
        w_r = [w2[k // 2][:, k % 2, :] for k in range(KT)]

        # ---- constants --------------------------------------------------
        ident = cpool.tile([128, 128], fp32)
        row_i = cpool.tile([128, 128], i32)
        col_i = cpool.tile([128, 128], i32)
        nc.gpsimd.iota(row_i[:], pattern=[[0, 128]], base=0, channel_multiplier=1)
        nc.gpsimd.iota(col_i[:], pattern=[[1, 128]], base=0, channel_multiplier=0)
        nc.vector.tensor_tensor(ident[:], row_i[:], col_i[:], Alu.is_equal)

        ones1 = cpool.tile([1, 128], fp32)
        nc.gpsimd.memset(ones1[:], 1.0)
        ones1r = cpool.tile([1, 128], f32r if use_f32r else fp32)
        nc.vector.tensor_copy(ones1r[:], ones1[:])  # memset can't write f32r
        ones4 = cpool.tile([4, 128], fp32)
        nc.gpsimd.memset(ones4[:], 1.0)
        ones4r = cpool.tile([4, 128], f32r if use_f32r else fp32)
        nc.vector.tensor_copy(ones4r[:], ones4[:])
        maskL = cpool.tile([4, 4], fp32)  # maskL[j,l] = 1 if j < l
        nc.vector.tensor_tensor(maskL[:], row_i[0:4, 0:4], col_i[0:4, 0:4], Alu.is_lt)

        # ---- Wc/Bs^T tiles [128(h), 4] via PE transpose -----------------
        wc_sb, bst_sb = [], []
        for k in range(KT):
            tp = pst.tile([128, 128], fp32, tag="tp")
            nc.tensor.transpose(
                tp[:, 0:4], ws_sb[:, k * 128 : (k + 1) * 128], ident[0:4, 0:4]
            )
            wck = smp.tile([128, 4], fp32, tag="wc")
            nc.scalar.copy(wck[:], tp[:, 0:4])
            wc_sb.append(wck)
        for k in range(KT):
            tp = pst.tile([128, 128], fp32, tag="tp")
            nc.tensor.transpose(
                tp[:, 0:4], bs_sb[:, k * 128 : (k + 1) * 128], ident[0:4, 0:4]
            )
            bsk = smp.tile([128, 4], fp32, tag="bst")
            nc.scalar.copy(bsk[:], tp[:, 0:4])
            bst_sb.append(bsk)

        # ---- Q = Bs^T.T @ Wc -> q_l = sum_{j<l} Q[j,l] ------------------
        q_ps = psq.tile([4, 4], fp32, tag="q")
        for k in range(KT):
            nc.tensor.matmul(
                q_ps[:], bst_sb[k][:], wc_sb[k][:], start=(k == 0), stop=(k == KT - 1)
            )
        qm_sb = cpool.tile([4, 4], fp32)
        nc.vector.tensor_tensor(qm_sb[:], q_ps[:], maskL[:], Alu.mult)
        qrow_ps = psq.tile([1, 4], fp32, tag="q")
        nc.tensor.matmul(qrow_ps[:], ones4[:, 0:1], qm_sb[:], start=True, stop=True)
        qrow_sb = cpool.tile([1, 4], fp32)
        nc.scalar.copy(qrow_sb[:], qrow_ps[:])
        qb_ps = psq.tile([128, 4], fp32, tag="q")
        nc.tensor.matmul(qb_ps[:], ones1[:], qrow_sb[:], start=True, stop=True)

        # bs rounded for the f32r B4 broadcast matmuls (emitted post-k-loop)
        bs_r = iop.tile([DEPTH, H], f32r if use_f32r else fp32)
        nc.vector.tensor_copy(bs_r[:], bs_sb[:])

        # ---- x^T tiles via PE transpose ---------------------------------
        xt_sb = []
        for k in range(KT):
            tp = pst.tile([128, 128], fp32, tag="tp")
            nc.tensor.transpose(tp[:], x_sb[:, k * 128 : (k + 1) * 128], ident[:])
            xtk = xtp.tile([128, 128], f32r if use_f32r else fp32, tag="xt")
            nc.vector.tensor_copy(xtk[:], tp[:])
            xt_sb.append(xtk)

        # ---- big matmul h = x @ W + be (k-outer, n-inner) ---------------
        h_sb = iop.tile([BS, H], fp32)
        out_sb = iop.tile([BS, H], fp32)
        c_sb = cpool.tile([128, 4], fp32)

        h_ps = [psh.tile([128, 512], fp32, tag="hps", name=f"hps{n}") for n in range(NT)]
        for n in range(NT):  # bias first: only needs be_sb, starts the group
            nc.tensor.matmul(
                h_ps[n][:], ones1r[:], be_sb[:, n * 512 : (n + 1) * 512],
                start=True, stop=False,
            )
        for k in range(KT - 2):
            for n in range(NT):
                nc.tensor.matmul(
                    h_ps[n][:], xt_sb[k][:], w_r[k][:, n * 512 : (n + 1) * 512],
                    start=False, stop=False,
                )
        # last chunk: finish half 0 first so its h-copies and transposes
        # overlap half 1's matmuls instead of trailing them
        for n in range(NT):
            for k in (KT - 2, KT - 1):
                nc.tensor.matmul(
                    h_ps[n][:], xt_sb[k][:], w_r[k][:, n * 512 : (n + 1) * 512],
                    start=False, stop=(k == KT - 1),
                )

        # ---- tail pipeline per 128-col tile: h copy -> h^T -> P matmul --
        # Pt[4,128] accumulates with the 4-column Wc as stationary operand
        # (LDWEIGHTS cost scales with stationary columns: ~free vs 128-col),
        # then one small transpose yields P^T[128,4]. Copies alternate
        # ACT/DVE so neither engine serializes the chain.
        pt4_ps = psq.tile([4, 128], fp32, tag="pt")
        for j in range(KT):
            n, c0 = j // 4, (j % 4) * 128
            if j % 2 == 0:
                nc.scalar.copy(
                    h_sb[:, j * 128 : (j + 1) * 128], h_ps[n][:, c0 : c0 + 128]
                )
            else:
                nc.vector.tensor_copy(
                    h_sb[:, j * 128 : (j + 1) * 128], h_ps[n][:, c0 : c0 + 128]
                )
            tp = pst.tile([128, 128], fp32, tag="tp", name=f"htp{j}")
            nc.tensor.transpose(tp[:], h_sb[:, j * 128 : (j + 1) * 128], ident[:])
            htj = htp.tile([128, 128], fp32, tag="ht", name=f"ht{j}")
            if j % 2 == 0:
                nc.vector.tensor_copy(htj[:], tp[:])
            else:
                nc.scalar.copy(htj[:], tp[:])
            nc.tensor.matmul(
                pt4_ps[:], wc_sb[j][:], htj[:],
                start=(j == 0), stop=(j == KT - 1),
                skip_group_check=True,
            )

        # ---- B4 broadcast rows (f32r: cheap) ----------------------------
        b4_ps = []
        for n in range(NT):
            b4 = psb.tile([128, 512], fp32, tag="b4", name=f"b4ps{n}")
            nc.tensor.matmul(
                b4[:], ones4r[:], bs_r[:, n * 512 : (n + 1) * 512],
                start=True, stop=True,
            )
            b4_ps.append(b4)

        pt4_sb = cpool.tile([4, 128], fp32)
        nc.scalar.copy(pt4_sb[:], pt4_ps[:])
        pt_ps = psq.tile([128, 4], fp32, tag="pt")
        nc.tensor.transpose(pt_ps[:], pt4_sb[:], ident[0:4, 0:4])

        # ---- c scan: c_{l+1} = (1 + P_l) * c_l + q_l --------------------
        at_sb = cpool.tile([128, 4], fp32)
        nc.vector.tensor_scalar_add(at_sb[:], pt_ps[:], 1.0)
        nc.vector.tensor_tensor_scan(
            c_sb[:], at_sb[:], qb_ps[:], 1.0, Alu.mult, Alu.add
        )

        # ---- final out = x0 * c4 + B4, per half, overlap DMA ------------
        for n in range(NT):
            nc.vector.scalar_tensor_tensor(
                out_sb[:, n * 512 : (n + 1) * 512],
                h_sb[:, n * 512 : (n + 1) * 512],
                c_sb[:, 3:4],
                b4_ps[n][:],
                Alu.mult,
                Alu.add,
            )
            # ACT ring: SP is busy with completion waits at this point
            nc.scalar.dma_start(
                y_out[:, n * 512 : (n + 1) * 512], out_sb[:, n * 512 : (n + 1) * 512]
            )

    if split:
        _split_multi_waits(nc)
    return nc


def kernel(x, W_enc, b_enc, ws, bs):
    from concourse.bass_utils import run_bass_kernel_spmd

    if "nc" not in _cache:
        _cache["nc"] = _build()
    nc = _cache["nc"]

    x = np.ascontiguousarray(x, dtype=np.float32)
    in_maps = []
    for c in range(N_CORES):
        in_maps.append(
            {
                "x": x[c * BS : (c + 1) * BS],
                "w": np.ascontiguousarray(W_enc, dtype=np.float32),
                "be": np.ascontiguousarray(b_enc, dtype=np.float32).reshape(1, H),
                "ws": np.ascontiguousarray(ws, dtype=np.float32).reshape(DEPTH, H),
                "bs": np.ascontiguousarray(bs, dtype=np.float32).reshape(DEPTH, H),
            }
        )
    res = run_bass_kernel_spmd(nc, in_maps, list(range(N_CORES)))
    return np.concatenate([res.results[c]["y"] for c in range(N_CORES)], axis=0)



# revision 5
# speedup vs baseline: 1.3328x; 1.3328x over previous
"""CrossNet layer (encoder Dense + 4 cross layers) on 8 trn2 NeuronCores.

Pure data parallelism: batch 1024 split into 8 shards of 128 rows; encoder
weights + tiny cross weights replicated per core.

Math: with h = x @ W_enc + b_enc, x0 = h, the cross recurrence
    x_{l+1} = x_l + x0 * (x_l @ w_l) + b_l
has closed form x_l = x0 * c_l + B_l with per-row scalar c_l and
B_l = sum_{j<l} b_j, via
    p_l = x0 @ w_l,  q_l = sum_{j<l} (b_j @ w_l),
    c_{l+1} = c_l * (1 + p_l) + q_l,  c_0 = 1,
so out = x0 * c_4 + B_4.

v2 layout strategy (vs the 40us fp32 baseline):
  - x arrives HOST-pre-transposed and bf16: xt[p, 128k+b] = x[b, 128k+p],
    so the k-stationary tiles DMA straight into place (no PE transposes).
  - W arrives bf16 (halves the dominant 2MB/core DMA stream) in
    column-half-major chunk order so the h->h^T->P tail for columns 0:512
    overlaps the second half of the W stream.
  - ws/bs arrive both pre-transposed ([H,4] fp32, for Wc/Bs^T tiles and the
    Q table) and as bf16 rows (for the B4 broadcast matmul); identity and
    ones come from host constants. No iota/memset/transpose prep at all.
  - h^T tail runs in f32r (1 cycle/col on PE vs 4 for fp32).
  - final out = x0*c4 + B4 as 4 quarter STTs so stores stream early.
"""

import numpy as np
import ml_dtypes

B, D, H, DEPTH = 1024, 1024, 1024, 4
N_CORES = 8
BS = B // N_CORES  # 128 batch rows per core
KT = D // 128      # 8 contraction k-tiles
NT = H // 512      # 2 psum column halves

BF16 = ml_dtypes.bfloat16

_cache = {}


def _patch_tile_drain(max_waits: int = 1):
    """walrus in this image allows only 1 sync-wait per instruction; the stock
    Tile end-of-kernel drain carries the whole global clock on one SP Drain and
    codegen fails. Split the waits across a chain of SP nops instead."""
    import concourse.tile as tile
    from concourse.vector_clock import ScopedClock
    from concourse import mybir

    if getattr(tile.TileContext, "_drain_patched", False):
        return

    def _drain_and_barrier(self, tick_clock, wait_clock):
        nc = self.nc
        carrier = nc.sync.nop()
        wait_clock.add_sem_waits(
            carrier.ins, ScopedClock({None: tick_clock.global_clock})
        )
        si = carrier.ins.sync_info
        if si is not None and si.on_wait and len(si.on_wait) > max_waits:
            waits = list(si.on_wait)
            carrier.ins.sync_info = mybir.SyncInfo(
                on_wait=waits[:max_waits], on_update=list(si.on_update or [])
            )
            rest = waits[max_waits:]
            while rest:
                extra = nc.sync.nop()
                extra.ins.sync_info = mybir.SyncInfo(
                    on_wait=rest[:max_waits], on_update=[]
                )
                rest = rest[max_waits:]
        nc.sync.drain()

        # exit barrier + sem clears dropped: the NEFF preamble re-inits
        # semaphores on every execution (verified by back-to-back runs), so
        # the ~4us exit butterfly only burns measured time
        assert self.sems is not None
        popped = nc._tile_sem_poison_stack.pop()
        assert popped is self._sem_poison
    tile.TileContext._drain_and_barrier = _drain_and_barrier
    tile.TileContext._drain_patched = True


def _split_multi_waits(nc):
    """walrus here allows only one sync-wait per instruction: move extra waits
    onto same-engine NoOps inserted immediately before the instruction."""
    from concourse import mybir

    for fn in nc.m.functions:
        for bb in fn.blocks:
            out = []
            for inst in bb.instructions:
                si = inst.sync_info
                if si is not None and si.on_wait and len(si.on_wait) > 1:
                    waits = list(si.on_wait)
                    for i, w in enumerate(waits[:-1]):
                        nop = mybir.InstNoOp(name=f"{inst.name}-w{i}", ins=[], outs=[])
                        nop.engine = inst.engine
                        nop.sync_info = mybir.SyncInfo(on_wait=[w], on_update=[])
                        out.append(nop)
                    inst.sync_info = mybir.SyncInfo(
                        on_wait=[waits[-1]], on_update=list(si.on_update or [])
                    )
                out.append(inst)
            bb.instructions[:] = out


def _build(split=True):
    from contextlib import ExitStack

    import concourse.bass as bass
    import concourse.tile as tile
    from concourse import mybir

    _patch_tile_drain()

    fp32 = mybir.dt.float32
    f32r = mybir.dt.float32r
    bf16 = mybir.dt.bfloat16
    Alu = mybir.AluOpType

    nc = bass.Bass()
    xt_in = nc.declare_dram_parameter("xt", [128, D], bf16, isOutput=False)
    w_in = nc.declare_dram_parameter("w", [D, H], bf16, isOutput=False)
    # cf32: wst [H,4] k-tiled | bst [H,4] k-tiled | maskL(j<l) | eye(4)
    cf32_in = nc.declare_dram_parameter("cf32", [128, 72], fp32, isOutput=False)
    # cbf: bf16 identity (transposes) | ones block (broadcast matmul stationaries)
    cbf_in = nc.declare_dram_parameter("cbf", [128, 256], bf16, isOutput=False)
    # sbf: be row (partition 0) | bs rows
    sbf_in = nc.declare_dram_parameter("sbf", [4, 2 * H], bf16, isOutput=False)
    y_out = nc.declare_dram_parameter("y", [BS, H], fp32, isOutput=True)

    with ExitStack() as ctx:
        tc = ctx.enter_context(tile.TileContext(nc))
        cpool = ctx.enter_context(tc.tile_pool(name="const", bufs=1))
        wpool = ctx.enter_context(tc.tile_pool(name="w", bufs=2 * KT))
        iop = ctx.enter_context(tc.tile_pool(name="io", bufs=1))
        htp = ctx.enter_context(tc.tile_pool(name="ht", bufs=KT))
        pst = ctx.enter_context(tc.tile_pool(name="pst", bufs=2, space="PSUM"))
        psh = ctx.enter_context(tc.tile_pool(name="psh", bufs=2, space="PSUM"))
        psb = ctx.enter_context(tc.tile_pool(name="psb", bufs=2, space="PSUM"))
        psq = ctx.enter_context(tc.tile_pool(name="psq", bufs=1, space="PSUM"))

        # ---- input DMAs -------------------------------------------------
        # ACT ring: xt first (gates every matmul), then the f32 consts.
        xt_sb = iop.tile([128, D], bf16)
        nc.scalar.dma_start(xt_sb[:], xt_in[:])
        cf32_sb = iop.tile([128, 72], fp32)
        nc.scalar.dma_start(cf32_sb[:], cf32_in[:])
        # gpsimd (SWDGE) ring: bf16 consts, parallel to ACT issue
        cbf_sb = cpool.tile([128, 256], bf16)
        nc.gpsimd.dma_start(cbf_sb[:], cbf_in[:])
        sbf_sb = cpool.tile([4, 2 * H], bf16)
        nc.gpsimd.dma_start(sbf_sb[:], sbf_in[:])
        # sync ring: the 2MB bf16 W stream, column-half-major
        w_t = []
        for n in range(NT):
            for cc in range(KT // 2):
                wt = wpool.tile([128, 2, 512], bf16, tag="w", name=f"w{n}{cc}")
                nc.sync.dma_start(
                    wt[:],
                    w_in[cc * 256 : (cc + 1) * 256, n * 512 : (n + 1) * 512].rearrange(
                        "(a p) h -> p a h", p=128
                    ),
                )
                w_t.append(wt)

        # ---- const views ------------------------------------------------
        wst = cf32_sb[:, 0:32]    # [128, (k l)] Wc k-tiles
        bst = cf32_sb[:, 32:64]   # [128, (k l)] Bs^T k-tiles
        maskL = cf32_sb[0:4, 64:68]
        eye4 = cf32_sb[0:4, 68:72]
        identb = cbf_sb[:, 0:128]
        ones1b = cbf_sb[0:1, 128:256]  # [1, 128] bf16 ones
        ones4b = cbf_sb[0:4, 128:256]  # [4, 128] bf16 ones
        be_row = sbf_sb[0:1, 0:H]
        bs_rows = sbf_sb[0:4, H : 2 * H]

        # ---- PSUM tiles -------------------------------------------------
        h_ps = [psh.tile([128, 512], fp32, tag="h", name=f"hps{n}") for n in range(NT)]
        b4_ps = [psb.tile([128, 512], fp32, tag="b4", name=f"b4ps{n}") for n in range(NT)]

        # bias opens each h accumulation group: h = be + sum_k xt_k^T @ W_k
        for n in range(NT):
            nc.tensor.matmul(
                h_ps[n][:], ones1b, be_row[:, n * 512 : (n + 1) * 512],
                start=True, stop=False,
            )

        # Q table: Q[j,l] = b_j @ w_l via Bs^T/Wc k-tiles
        q_ps = psq.tile([4, 4], fp32, tag="q")
        for k in range(KT):
            nc.tensor.matmul(
                q_ps[:],
                bst[:, 4 * k : 4 * k + 4],
                wst[:, 4 * k : 4 * k + 4],
                start=(k == 0), stop=(k == KT - 1),
            )
        qm_sb = cpool.tile([4, 4], bf16)
        nc.vector.tensor_tensor(qm_sb[:], q_ps[:], maskL, Alu.mult)
        qrow_sb = cpool.tile([1, 4], bf16)
        wcb = cpool.tile([128, 32], bf16)  # Wc k-tiles, bf16 for the P matmuls
        nc.vector.tensor_copy(wcb[:], wst)

        hb = iop.tile([128, H], bf16)      # h, bf16, feeds the h^T transposes
        b4_sb = iop.tile([128, H], fp32)   # B4 rows (SBUF so the STT reads h from PSUM)
        out_sb = iop.tile([128, H], fp32)
        pt4_ps = psq.tile([4, 128], fp32, tag="pt")

        def emit_chunk_mms(n, cc):
            for a in range(2):
                k = 2 * cc + a
                nc.tensor.matmul(
                    h_ps[n][:],
                    xt_sb[:, 128 * k : 128 * (k + 1)],
                    w_t[n * 4 + cc][:, a, :],
                    start=False, stop=(cc == 3 and a == 1),
                )

        def emit_tail_copy(j):
            n, c0 = j // 4, (j % 4) * 128
            src = h_ps[n][:, c0 : c0 + 128]
            dst = hb[:, 128 * j : 128 * (j + 1)]
            if j % 2 == 0:
                nc.scalar.copy(dst, src)
            else:
                nc.vector.tensor_copy(dst, src)

        tp_tiles = {}

        def emit_tail_pe(j):
            tp = pst.tile([128, 128], bf16, tag="tp", name=f"tp{j}")
            nc.tensor.transpose(tp[:], hb[:, 128 * j : 128 * (j + 1)], identb)
            tp_tiles[j] = tp
            htj = htp.tile([128, 128], bf16, tag="ht", name=f"ht{j}")
            if j % 2 == 0:
                nc.vector.tensor_copy(htj[:], tp[:])
            else:
                nc.scalar.copy(htj[:], tp[:])
            nc.tensor.matmul(
                pt4_ps[:],
                wcb[:, 4 * j : 4 * j + 4],
                htj[:],
                start=(j == 0), stop=(j == KT - 1),
                skip_group_check=True,
            )

        # ---- half 0 stream ----------------------------------------------
        emit_chunk_mms(0, 0)
        # qrow_l = sum_{j<l} Q[j,l]: colsum via ones, then broadcast to rows
        qrow_ps = psq.tile([1, 4], fp32, tag="q")
        nc.tensor.matmul(
            qrow_ps[:], ones4b[:, 0:1], qm_sb[:],
            start=True, stop=True, skip_group_check=True,
        )
        nc.scalar.copy(qrow_sb[:], qrow_ps[:])
        emit_chunk_mms(0, 1)
        qb_ps = psq.tile([128, 4], fp32, tag="q")
        nc.tensor.matmul(
            qb_ps[:], ones1b, qrow_sb[:],
            start=True, stop=True, skip_group_check=True,
        )
        emit_chunk_mms(0, 2)
        for n in range(NT):
            nc.tensor.matmul(
                b4_ps[n][:], ones4b, bs_rows[:, n * 512 : (n + 1) * 512],
                start=True, stop=True, skip_group_check=True,
            )
        for n in range(NT):
            nc.scalar.copy(b4_sb[:, n * 512 : (n + 1) * 512], b4_ps[n][:])
        emit_chunk_mms(0, 3)

        # tail copies for half 0 unlock as soon as h_ps[0] stops
        for j in range(4):
            emit_tail_copy(j)

        # ---- half 1 stream, h^T/P tail for half 0 interleaved ------------
        emit_chunk_mms(1, 0)
        emit_tail_pe(0)
        emit_chunk_mms(1, 1)
        emit_tail_pe(1)
        emit_chunk_mms(1, 2)
        emit_tail_pe(2)
        emit_chunk_mms(1, 3)
        emit_tail_pe(3)

        for j in range(4, KT):
            emit_tail_copy(j)
            emit_tail_pe(j)

        # ---- c scan: c_{l+1} = (1 + P_l) * c_l + q_l ---------------------
        pt4_sb = cpool.tile([4, 128], fp32)
        nc.scalar.copy(pt4_sb[:], pt4_ps[:])
        pt_ps = psq.tile([128, 4], fp32, tag="pt")
        nc.tensor.transpose(pt_ps[:], pt4_sb[:], eye4)
        at_sb = cpool.tile([128, 4], fp32)
        nc.vector.tensor_scalar_add(at_sb[:], pt_ps[:], 1.0)
        c_sb = cpool.tile([128, 4], fp32)
        nc.vector.tensor_tensor_scan(
            c_sb[:], at_sb[:], qb_ps[:], 1.0, Alu.mult, Alu.add
        )

        # ---- out = x0 * c4 + B4 per quarter, stores stream on sync ring --
        for qq in range(4):
            n, c0 = qq // 2, (qq % 2) * 256
            nc.vector.scalar_tensor_tensor(
                out_sb[:, qq * 256 : (qq + 1) * 256],
                h_ps[n][:, c0 : c0 + 256],
                c_sb[:, 3:4],
                b4_sb[:, qq * 256 : (qq + 1) * 256],
                Alu.mult,
                Alu.add,
            )
            eng = nc.sync if qq % 2 == 0 else nc.scalar
            eng.dma_start(
                y_out[:, qq * 256 : (qq + 1) * 256],
                out_sb[:, qq * 256 : (qq + 1) * 256],
            )

    if split:
        _split_multi_waits(nc)
    return nc


def prep_in_maps(x, W_enc, b_enc, ws, bs):
    """Host-side sharding prep: layout + dtype only (no model arithmetic)."""
    x = np.ascontiguousarray(x, dtype=np.float32)
    ws2 = np.asarray(ws, dtype=np.float32).reshape(DEPTH, H)
    bs2 = np.asarray(bs, dtype=np.float32).reshape(DEPTH, H)

    w_bf = np.ascontiguousarray(W_enc, dtype=np.float32).astype(BF16)

    cf32 = np.zeros((128, 72), dtype=np.float32)
    cf32[:, 0:32] = ws2.T.reshape(KT, 128, DEPTH).transpose(1, 0, 2).reshape(128, 32)
    cf32[:, 32:64] = bs2.T.reshape(KT, 128, DEPTH).transpose(1, 0, 2).reshape(128, 32)
    jj, ll = np.indices((DEPTH, DEPTH))
    cf32[0:4, 64:68] = (jj < ll).astype(np.float32)
    cf32[0:4, 68:72] = np.eye(4, dtype=np.float32)

    cbf = np.ones((128, 256), dtype=np.float32)
    cbf[:, 0:128] = np.eye(128, dtype=np.float32)
    cbf = cbf.astype(BF16)

    sbf = np.zeros((4, 2 * H), dtype=np.float32)
    sbf[0, 0:H] = np.asarray(b_enc, dtype=np.float32).reshape(H)
    sbf[:, H : 2 * H] = bs2
    sbf = sbf.astype(BF16)

    in_maps = []
    for c in range(N_CORES):
        xc = x[c * BS : (c + 1) * BS]  # [128, 1024]
        # xt[p, 128k + b] = x[b, 128k + p]
        xt = np.ascontiguousarray(
            xc.reshape(BS, KT, 128).transpose(2, 1, 0).reshape(128, D)
        ).astype(BF16)
        in_maps.append(
            {"xt": xt, "w": w_bf, "cf32": cf32, "cbf": cbf, "sbf": sbf}
        )
    return in_maps


def kernel(x, W_enc, b_enc, ws, bs):
    from concourse.bass_utils import run_bass_kernel_spmd

    if "nc" not in _cache:
        _cache["nc"] = _build()
    nc = _cache["nc"]

    in_maps = prep_in_maps(x, W_enc, b_enc, ws, bs)
    res = run_bass_kernel_spmd(nc, in_maps, list(range(N_CORES)))
    return np.concatenate([res.results[c]["y"] for c in range(N_CORES)], axis=0)
